# revision 12
# baseline (speedup 1.0000x reference)
"""KNN-conv kernel for Trainium2, data-parallel over batch on 8 NeuronCores.

Problem: for x (32, 128, 32, 32) and conv weight W (128, 128, 9):
  per batch: cosine-sim (1024x1024) over channels, diag -> +INF, top-9
  neighbors per token, gather neighbor features, contract with W.

v2 design (vs. the 299us baseline):
  - normalization moved to host: device receives xn pre-normalized,
    eliminating the on-device norm/reciprocal/broadcast chain.
  - similarity either fp32r single-pass (KNN_SIM=f32r) or 3-pass bf16
    hi/lo split (hi.hi + hi.lo + lo.hi) at 1 cyc/row, vs. baseline's
    true-fp32 at 4 cyc/row.
  - self-similarity suppressed to -1e10 via an accumulating ident x negi
    matmul; rank-0 neighbor (self) handled as conv k=0 from a plain load.
  - top-8: max8 + max_index read the sim PSUM tile directly (no SBUF copy).
  - index fold to the gather's 16-wrapped layout is done with 8 tiny DMAs
    per batch at 16-byte granularity by exploiting a free position
    permutation pi (c<->sl digit swap, an involution): positions are
    pi-permuted on device and the host un-permutes the output columns.
    (The baseline's 2-byte-granule fold DMAs cost ~115us.)
  - the 8x16-partition replication the SWDGE idx contract needs is done
    with one fp32r matmul against a 0/1 block matrix (exact for integer
    values) instead of 8 DMAs.
  - neighbor features gathered once per batch (8 ranks, 8192 idxs) as fp16
    (256B/token); conv accumulates 9 fp16 matmul pairs into one PSUM tile.
"""

import os

import numpy as np

B, C, N, K = 32, 128, 32 * 32, 9
O = 128  # out channels
NCORES = 8
BPC = B // NCORES  # batches per core
NEG = -1.0e10

_prog_cache = {}
last_results = None  # BassKernelResults of the most recent run (for test.py)


def _perm():
    """pi(i) = 128*(i//16 % 8) + 16*(i//128) + i%16  (involution)."""
    i = np.arange(N)
    return (128 * ((i // 16) % 8) + 16 * (i // 128) + (i % 16)).astype(np.int64)


def _sim_mode():
    return os.environ.get("KNN_SIM", "bf3")


def _build_program():
    import concourse.bacc as bacc
    import concourse.mybir as mybir
    from concourse.tile import TileContext

    f32 = mybir.dt.float32
    f32r = mybir.dt.float32r
    bf16 = mybir.dt.bfloat16
    fp16 = mybir.dt.float16
    u16 = mybir.dt.uint16
    i16 = mybir.dt.int16
    AF = mybir.ActivationFunctionType

    sim_mode = _sim_mode()
    nc = bacc.Bacc()

    if sim_mode == "f32r":
        xns_h = nc.declare_dram_parameter("xns", [BPC, C, N], f32, isOutput=False)
    else:
        xns_h = nc.declare_dram_parameter("xns", [BPC, 2, C, N], bf16, isOutput=False)
    xt_h = nc.declare_dram_parameter("xt", [BPC, N, C], fp16, isOutput=False)
    xcm_h = nc.declare_dram_parameter("xcm", [BPC, C, N], fp16, isOutput=False)
    wt_h = nc.declare_dram_parameter("wt", [C, K * O], fp16, isOutput=False)
    ident_h = nc.declare_dram_parameter("ident", [128, 128], bf16, isOutput=False)
    negi_h = nc.declare_dram_parameter("negi", [128, 128], bf16, isOutput=False)
    repi_h = nc.declare_dram_parameter("repi", [16, 128], fp16, isOutput=False)
    out_h = nc.declare_dram_parameter("out", [BPC, O, N], f32, isOutput=True)

    with TileContext(nc) as tc:
        with (
            tc.tile_pool(name="consts", bufs=1) as consts,
            tc.tile_pool(name="xnp", bufs=int(os.environ.get("KNN_XNP", "2"))) as xnp,
            tc.tile_pool(name="xcp", bufs=2) as xcp,
            tc.tile_pool(name="v8p", bufs=2) as v8p,
            tc.tile_pool(name="idxp", bufs=2) as idxp,
            tc.tile_pool(name="g16p", bufs=2) as g16p,
            tc.tile_pool(name="gallp", bufs=2) as gallp,
            tc.tile_pool(name="prp", bufs=2) as prp,
            tc.tile_pool(name="outp", bufs=2) as outp,
            tc.tile_pool(name="psb", bufs=int(os.environ.get("KNN_PSB", "2")),
                         space="PSUM") as psb,
            tc.tile_pool(name="pso", bufs=1, space="PSUM") as pso,
            tc.tile_pool(name="psr", bufs=1, space="PSUM") as psr,
        ):
            wts = consts.tile([C, K * O], fp16, tag="wts")
            nc.sync.dma_start(out=wts[:], in_=wt_h[:])
            ident = consts.tile([128, 128], bf16, tag="ident")
            nc.sync.dma_start(out=ident[:], in_=ident_h[:])
            negi = consts.tile([128, 128], bf16, tag="negi")
            nc.sync.dma_start(out=negi[:], in_=negi_h[:])
            repi = consts.tile([16, 128], fp16, tag="repi")
            nc.sync.dma_start(out=repi[:], in_=repi_h[:])

            for b in range(BPC):
                # ---- load sim operand and self features -------------------
                if sim_mode == "f32r":
                    XN = xnp.tile([C, N], f32, tag="xn")
                    nc.sync.dma_start(out=XN[:], in_=xns_h[b])
                else:
                    XN = xnp.tile([C, 2 * N], bf16, tag="xn")  # [hi | lo]
                    nc.sync.dma_start(
                        out=XN[:].rearrange("c (t n) -> c t n", t=2),
                        in_=xns_h[b].rearrange("t c n -> c t n"),
                    )
                XCM = xcp.tile([C, N], fp16, tag="xcm")  # pi-permuted self
                nc.sync.dma_start(out=XCM[:], in_=xcm_h[b])

                # ---- similarity + top-8 -----------------------------------
                IDX = idxp.tile([128, 64], u16, tag="idx")
                for c in range(8):
                    ps = psb.tile([128, N], f32, tag="ps_sim")
                    dh = 0 if c < 4 else 1  # half containing the diag block
                    if sim_mode == "f32r":
                        blk = XN[:, c * 128 : (c + 1) * 128].bitcast(f32r)
                        for h in range(2):
                            cols = slice(h * 512, (h + 1) * 512)
                            nc.tensor.matmul(
                                ps[:, cols], blk, XN[:, cols].bitcast(f32r),
                                start=True, stop=(h != dh),
                            )
                    else:
                        hi = XN[:, 0:N]
                        lo = XN[:, N : 2 * N]
                        hi_blk = hi[:, c * 128 : (c + 1) * 128]
                        lo_blk = lo[:, c * 128 : (c + 1) * 128]
                        for h in range(2):
                            cols = slice(h * 512, (h + 1) * 512)
                            nc.tensor.matmul(
                                ps[:, cols], hi_blk, hi[:, cols],
                                start=True, stop=False,
                            )
                            nc.tensor.matmul(
                                ps[:, cols], hi_blk, lo[:, cols],
                                start=False, stop=False,
                            )
                            nc.tensor.matmul(
                                ps[:, cols], lo_blk, hi[:, cols],
                                start=False, stop=(h != dh),
                            )
                    # diag block -> -1e10 (accumulate ident.T @ negi)
                    nc.tensor.matmul(
                        ps[:, c * 128 : c * 128 + 128], ident[:], negi[:],
                        start=False, stop=True,
                    )
                    V8 = v8p.tile([128, 8], f32, tag="v8")
                    nc.vector.max(V8[:], ps[:])
                    # IDX[p, 8j+c] = rank-(j+1) neighbor of token c*128+p
                    nc.vector.max_index(IDX[:, c : 64 : 8], V8[:], ps[:])

                # ---- fold to 16-wrapped gather layout ---------------------
                # G16[q, 64j+8sl+c] = IDX[16sl+q, 8j+c]; 16B-granule DMAs
                G16 = g16p.tile([128, 512], u16, tag="g16")
                gout = G16[0:16, :].rearrange("q (jj rest) -> q jj rest", jj=8)
                for sl in range(8):
                    nc.scalar.dma_start(
                        out=gout[:, :, 8 * sl : 8 * sl + 8],
                        in_=IDX[16 * sl : 16 * sl + 16, :].rearrange(
                            "q (jj c) -> q jj c", jj=8
                        ),
                    )
                # replicate idx rows across all 8 16-partition groups (the
                # SWDGE lanes each read their own group): exact fp32r matmul
                # against a 0/1 block matrix, with int<->float converts.
                G16F = g16p.tile([16, 512], fp16, tag="g16f")
                nc.gpsimd.tensor_copy(G16F[:], G16[0:16, :])
                PSR = psr.tile([128, 512], f32, tag="ps_rep")
                nc.tensor.matmul(PSR[:], repi[:], G16F[:], start=True, stop=True)
                GALL = gallp.tile([128, 512], u16, tag="gall")
                nc.scalar.activation(GALL[:], PSR[:], AF.Copy)

                # ---- gather neighbor features (8 ranks, fp16) -------------
                PR = prp.tile([C, 8 * N], fp16, tag="pr")
                nc.gpsimd.dma_gather(
                    out_ap=PR[:].rearrange("p (one n) -> p one n", one=1),
                    in_ap=xt_h[b],
                    idxs_ap=GALL[:].bitcast(i16),
                    num_idxs=8 * N,
                    num_idxs_reg=8 * N,
                    elem_size=C,
                    transpose=True,
                    single_packet=False,
                )

                # ---- conv contraction (fp16, 9 taps) ----------------------
                PO = pso.tile([O, N], f32, tag="ps_out")
                for k in range(K):
                    w_k = wts[:, k * O : (k + 1) * O]
                    for h in range(2):
                        cols = slice(h * 512, (h + 1) * 512)
                        if k == 0:
                            src = XCM[:, cols]
                        else:
                            src = PR[:, (k - 1) * N + h * 512 : (k - 1) * N + (h + 1) * 512]
                        nc.tensor.matmul(
                            PO[:, cols], w_k, src, start=(k == 0), stop=(k == K - 1)
                        )
                OUT = outp.tile([O, N], f32, tag="out")
                nc.scalar.activation(OUT[:], PO[:], AF.Copy)
                nc.sync.dma_start(out=out_h[b], in_=OUT[:])

    nc.compile()
    return nc


def _get_program():
    key = _sim_mode()
    if key not in _prog_cache:
        _prog_cache[key] = _build_program()
    return _prog_cache[key]


def _host_prep(x, W):
    """Build per-core input maps from full inputs."""
    import ml_dtypes

    bf16 = ml_dtypes.bfloat16
    fp16 = np.float16

    xf = np.ascontiguousarray(x.reshape(B, C, N).astype(np.float32, copy=False))
    norm = np.linalg.norm(xf, axis=1, keepdims=True)
    xn = (xf / (norm + 1e-8)).astype(np.float32)

    if _sim_mode() == "f32r":
        xns = xn
    else:
        hi = xn.astype(bf16)
        lo = (xn - hi.astype(np.float32)).astype(bf16)
        xns = np.stack([hi, lo], axis=1)  # (B, 2, C, N)

    x16 = xf.astype(fp16)
    xt = np.ascontiguousarray(x16.transpose(0, 2, 1))  # (B, N, C) token-major

    perm = _perm()
    xcm = np.ascontiguousarray(x16[:, :, perm])  # (B, C, N) pi-permuted

    wt = np.ascontiguousarray(
        np.transpose(W.astype(np.float32, copy=False), (1, 2, 0))
    ).reshape(C, K * O).astype(fp16)

    ident = np.eye(128, dtype=bf16)
    negi = (NEG * np.eye(128, dtype=np.float32)).astype(bf16)
    repi = np.zeros((16, 128), np.float16)
    repi[np.arange(128) % 16, np.arange(128)] = 1.0

    in_maps = []
    for i in range(NCORES):
        sl = slice(i * BPC, (i + 1) * BPC)
        in_maps.append(
            {
                "xns": np.ascontiguousarray(xns[sl]),
                "xt": np.ascontiguousarray(xt[sl]),
                "xcm": np.ascontiguousarray(xcm[sl]),
                "wt": wt,
                "ident": ident,
                "negi": negi,
                "repi": repi,
            }
        )
    return in_maps


def kernel(x, W):
    global last_results
    from concourse.bass_utils import run_bass_kernel_spmd

    x = np.asarray(x)
    W = np.asarray(W)
    in_maps = _host_prep(x, W)
    nc = _get_program()
    trace = bool(int(os.environ.get("KNN_TRACE", "0")))
    res = run_bass_kernel_spmd(nc, in_maps, list(range(NCORES)), trace=trace)
    last_results = res
    out = np.concatenate([res.results[i]["out"] for i in range(NCORES)], axis=0)
    out = out[:, :, _perm()]  # un-permute positions (pi is an involution)
    return out.reshape(B, O, 32, 32).astype(np.float32, copy=False)
